# revision 9
# baseline (speedup 1.0000x reference)
"""Multi-head attention (B=2, S=2048, H=1024, 16 heads) on 8 trn2 NeuronCores.

Sharding: batch(2) x head-group(4) tensor parallel. Core (b, g) owns batch b
and heads 4g..4g+3 (channels 256g..256g+256 of the QKV projections / input
channels of the output projection). Partial wo outputs are summed on host.

Device-side dataflow per core (matmuls bf16, f32 PSUM accumulation):
  QT/KT[c, s]: transposed projections (channels on partitions); bias fused
  into the PSUM-drain copy (tensor_scalar_add with a [128,1] bias column).
  V projected directly in natural layout [s, ch] (lhsT = x chunk), drained
  with a strided scalar_tensor_tensor into VA4[s, kk, head, 65] (col 64 is
  a ones column -> row sums ride the AV matmul).
  Per head-pair p, query-block qb (512 q), key-tile kk (128 k):
    sc[k, 0:512]=h_even scores, sc[k, 512:1024]=h_odd  (row-packed concurrent)
    e = exp(sc/8)  (single [128,1024] ACT instr, both heads)
    po[0:65, 0:512] += VA4[.,kk,even,:] . e_even ; po[:, 512:1024] += odd
  Epilogue: early-drain po->SBUF, DMA-repack of the sums row to [64,16] for
  a cheap DVE reciprocal, DMA back, gpsimd partition_broadcast, DVE muls.
  wo flipped: y[q, oc] = on_pair0.T @ wo0 + on_pair1.T @ wo1.
  Intro: DMA descriptor issue spread across all 5 engine queues (it
  serializes ~650ns/descriptor per queue); 12 warmup matmuls keep the PE
  HAM clock-gate warm before real work lands; wo(2) rides inside the last
  attention block so its normalization chain hides under matmuls.
"""

import os
import threading

import numpy as np
import ml_dtypes

import concourse.bass as bass
import concourse.mybir as mybir
import concourse.tile as tile
from concourse import bacc
from concourse.bass_utils import run_bass_kernel_spmd

BF16 = ml_dtypes.bfloat16
F32 = mybir.dt.float32
BF = mybir.dt.bfloat16

B = 2
S = 2048
H = 1024
NH = 16
HD = 64
NG = 4              # head groups (TP degree)
HPG = 4             # heads per group
CPG = HPG * HD      # 256 channels per group
NF = H // 128       # 8 input-feature chunks
N_CORES = 8
NKT = S // 128      # 16 key tiles
NQB = S // 512      # 4 query blocks
QB = 512

_cache = threading.Lock()
_nc = None

LAST_RESULT = None  # BassKernelResults of the most recent run (for test.py)


def _build_nc():
    nc = bacc.Bacc(None, target_bir_lowering=False, debug=False)

    xq_d = nc.dram_tensor("xq_t", [H, S], BF, kind="ExternalInput")
    xk_d = nc.dram_tensor("xk_t", [H, S], BF, kind="ExternalInput")
    xv_d = nc.dram_tensor("xv_t", [H, S], BF, kind="ExternalInput")
    wq_d = nc.dram_tensor("wq_t", [H, CPG], BF, kind="ExternalInput")
    wk_d = nc.dram_tensor("wk_t", [H, CPG], BF, kind="ExternalInput")
    wv_d = nc.dram_tensor("wv_t", [H, CPG], BF, kind="ExternalInput")
    bq_d = nc.dram_tensor("bq", [CPG, 1], F32, kind="ExternalInput")
    bk_d = nc.dram_tensor("bk", [CPG, 1], F32, kind="ExternalInput")
    bv_d = nc.dram_tensor("bv", [1, CPG], F32, kind="ExternalInput")
    wo_d = nc.dram_tensor("wo_t", [CPG, H], BF, kind="ExternalInput")
    y_d = nc.dram_tensor("y_t", [S, H], BF, kind="ExternalOutput")

    xq_ap = xq_d.rearrange("(nf p) s -> nf p s", p=128)
    xk_ap = xk_d.rearrange("(nf p) s -> nf p s", p=128)
    xv_ap = xv_d.rearrange("(nf p) s -> nf p s", p=128)
    y_ap = y_d.rearrange("(nt p) o -> nt p o", p=128)

    Exp = mybir.ActivationFunctionType.Exp
    MULT = mybir.AluOpType.mult
    ADD = mybir.AluOpType.add

    with tile.TileContext(nc) as tc:
        with (
            tc.tile_pool(name="const", bufs=1) as const,
            tc.tile_pool(name="xpool", bufs=8) as xpool,
            tc.tile_pool(name="res", bufs=1) as res,
            tc.tile_pool(name="epool", bufs=2) as epool,
            tc.tile_pool(name="npool", bufs=2) as npool,
            tc.tile_pool(name="ypool", bufs=3) as ypool,
            tc.tile_pool(name="psum", bufs=1, space="PSUM") as psum,
        ):
            # --- constants / weights ---
            wq_sb = const.tile([128, NF, CPG], BF)
            wk_sb = const.tile([128, NF, CPG], BF)
            wv_sb = const.tile([128, NF, CPG], BF)
            wo_sb = const.tile([128, 2, H], BF)
            bq_sb = const.tile([128, 2], F32)
            bk_sb = const.tile([128, 2], F32)
            bv_sb = const.tile([1, CPG], F32)
            bvb = const.tile([128, CPG], F32)
            warm = const.tile([128, QB], BF)

            # --- residents ---
            QT = [res.tile([128, S], BF, name=f"QT{p}") for p in range(2)]
            KT = [res.tile([128, S], BF, name=f"KT{p}") for p in range(2)]
            VA4 = res.tile([128, NKT, 4, HD + 1], BF, name="VA4")

            # --- input staging ---
            # DMA descriptor issue serializes ~650ns/descriptor per queue and
            # only sync/scalar/gpsimd can issue; the first attention block's
            # critical path is wq+wk+win0 of xq/xk (~3MB), so those are
            # win-split into their own tiles and issued first, spread across
            # the three queues; everything else streams behind.
            xq0_l = [xpool.tile([128, QB], BF, tag="xq0", name=f"xq0_{f}")
                     for f in range(NF)]
            xqr_l = [xpool.tile([128, 3 * QB], BF, tag="xqr", name=f"xqr{f}")
                     for f in range(NF)]
            xk0_l = [xpool.tile([128, QB], BF, tag="xk0", name=f"xk0_{f}")
                     for f in range(NF)]
            xkr_l = [xpool.tile([128, 3 * QB], BF, tag="xkr", name=f"xkr{f}")
                     for f in range(NF)]
            xv_l = [xpool.tile([128, S], BF, tag="xv", name=f"xv{f}")
                    for f in range(NF)]

            # gpsimd: memsets first (warmup + VA ones), then its DMA share
            nc.gpsimd.memset(warm[:], 0.125)
            nc.gpsimd.memset(VA4[:, :, :, HD : HD + 1], 1.0)
            nc.gpsimd.dma_start(wv_sb[:], wv_d.rearrange("(nf p) c -> p nf c", p=128))
            nc.gpsimd.dma_start(bv_sb[:], bv_d[:])
            nc.gpsimd.partition_broadcast(bvb[:], bv_sb[:])
            for f in range(NF):
                nc.gpsimd.dma_start(xv_l[f][:], xv_ap[f])
            nc.gpsimd.dma_start(wo_sb[:], wo_d.rearrange("(t p) o -> p t o", p=128))

            # sync: wq + biases + xq win0 + xq rest
            nc.sync.dma_start(wq_sb[:], wq_d.rearrange("(nf p) c -> p nf c", p=128))
            nc.sync.dma_start(bq_sb[:], bq_d.rearrange("(t p) a -> p (t a)", p=128))
            for f in range(NF // 2):
                nc.sync.dma_start(xq0_l[f][:], xq_ap[f][:, 0:QB])
            nc.sync.dma_start(bk_sb[:], bk_d.rearrange("(t p) a -> p (t a)", p=128))
            for f in range(NF // 2, NF):
                nc.sync.dma_start(xq0_l[f][:], xq_ap[f][:, 0:QB])
            for f in range(NF):
                nc.sync.dma_start(xqr_l[f][:], xq_ap[f][:, QB:S])

            # scalar: wk + xk win0, a dummy exp to pull ACT_TABLE_LOAD early,
            # then xk rest
            nc.scalar.dma_start(wk_sb[:], wk_d.rearrange("(nf p) c -> p nf c", p=128))
            for f in range(NF):
                nc.scalar.dma_start(xk0_l[f][:], xk_ap[f][:, 0:QB])
            dume = const.tile([1, 16], F32)
            nc.scalar.activation(dume[:], warm[0:1, 0:16], Exp, scale=1.0)
            for f in range(NF):
                nc.scalar.dma_start(xkr_l[f][:], xk_ap[f][:, QB:S])

            # --- PE warmup: keep HAM clock-gate busy until real work lands ---
            for i in range(12):
                pw = psum.tile([128, 256], F32, tag="py", name=f"warm{i}", bufs=2)
                nc.tensor.matmul(pw[:], lhsT=warm[:, 0:128], rhs=warm[:, 0:256],
                                 start=True, stop=True)

            def xap_split(t0, tr):
                def ap(f, win):
                    if win == 0:
                        return t0[f][:]
                    return tr[f][:, (win - 1) * QB:win * QB]
                return ap

            xq_t = xap_split(xq0_l, xqr_l)
            xk_t = xap_split(xk0_l, xkr_l)

            # ---- filler units: ~0.5-1.5us chunks of PE side-work that get
            # interleaved into the attention kk-loops (the static scheduler
            # follows emission order per engine, so coarse blocks stall ACT)
            _proj_state = {}

            def proj_half(p, xt, w_sb, b_sb, out_t, win, half):
                cs = slice(p * 128, (p + 1) * 128)
                ws = slice(win * QB, (win + 1) * QB)
                key = (id(out_t), win)
                if half == 0:
                    ps = psum.tile([128, QB], F32, tag="py",
                                   name=f"ps{p}{win}", bufs=2)
                    _proj_state[key] = ps
                    fs = range(0, NF // 2)
                else:
                    ps = _proj_state.pop(key)
                    fs = range(NF // 2, NF)
                for f in fs:
                    nc.tensor.matmul(
                        ps[:], lhsT=w_sb[:, f, cs], rhs=xt(f, win),
                        start=(f == 0), stop=(f == NF - 1),
                    )
                if half == 1:
                    nc.vector.tensor_scalar_add(
                        out_t[:, ws], ps[:], b_sb[:, p:p + 1])

            def qk_proj(p, xt, w_sb, b_sb, out_t, wins=None):
                for win in (range(S // QB) if wins is None else wins):
                    proj_half(p, xt, w_sb, b_sb, out_t, win, 0)
                    proj_half(p, xt, w_sb, b_sb, out_t, win, 1)

            def proj_units(p, xt, w_sb, b_sb, out_t, wins):
                return [
                    (lambda w=w, h=h: proj_half(p, xt, w_sb, b_sb, out_t, w, h))
                    for w in wins for h in (0, 1)
                ]

            # V in natural layout: pv[s, ch] = sum_f xv[f,s].T @ wv[f,ch],
            # drained (plus bias) straight into the augmented VA4 tile.
            def v_unit(st):
                tsl = slice(st * 128, (st + 1) * 128)
                pv = psum.tile([128, CPG], F32, tag="py",
                               name=f"pv{st}", bufs=2)
                for f in range(NF):
                    nc.tensor.matmul(
                        pv[:], lhsT=xv_l[f][:, tsl], rhs=wv_sb[:, f, :],
                        start=(f == 0), stop=(f == NF - 1),
                    )
                nc.vector.scalar_tensor_tensor(
                    VA4[:, st, :, 0:HD],
                    pv[:].rearrange("p (h d) -> p h d", h=4),
                    1.0,
                    bvb[:].rearrange("p (h d) -> p h d", h=4),
                    MULT, ADD,
                )

            def v_units(sts):
                return [(lambda s=s: v_unit(s)) for s in sts]

            on_tiles = {}
            filler = []

            def self_av(p, po, kk, e):
                nc.tensor.matmul(
                    po[0:HD + 1, 0:QB], lhsT=VA4[:, kk, 2 * p, :],
                    rhs=e[:, 0:QB],
                    start=(kk == 0), stop=(kk == NKT - 1),
                )
                nc.tensor.matmul(
                    po[0:HD + 1, QB:1024], lhsT=VA4[:, kk, 2 * p + 1, :],
                    rhs=e[:, QB:1024],
                    start=(kk == 0), stop=(kk == NKT - 1),
                )

            def attention(p, qb, split=False, pops=2):
                qs = slice(qb * QB, (qb + 1) * QB)
                po = psum.tile([128, 1024], F32, tag="po",
                               name=f"po{p}{qb}", bufs=1)
                es = []
                for kk in range(NKT):
                    for _ in range(pops):
                        if filler:
                            filler.pop(0)()
                    ks = slice(kk * 128, (kk + 1) * 128)
                    sc = psum.tile([128, 1024], F32, tag="sc",
                                   name=f"sc{p}{qb}{kk}", bufs=2)
                    nc.tensor.matmul(
                        sc[:, 0:QB],
                        lhsT=KT[p][0:HD, ks], rhs=QT[p][0:HD, qs],
                        tile_position=(0, 0),
                    )
                    nc.tensor.matmul(
                        sc[:, QB:1024],
                        lhsT=KT[p][HD:128, ks], rhs=QT[p][HD:128, qs],
                        tile_position=(64, 0),
                    )
                    e = epool.tile([128, 1024], BF, tag="e", name=f"e{p}{qb}{kk}",
                                   bufs=16)
                    nc.scalar.activation(e[:], sc[:], Exp, scale=0.125)
                    es.append(e)
                    if not split:
                        self_av(p, po, kk, es[kk])
                if split:
                    for kk in range(NKT):
                        if filler:
                            filler.pop(0)()
                        self_av(p, po, kk, es[kk])
                # epilogue: drain the sums row first (it gates the whole
                # reciprocal round-trip), then the po body; reciprocal of the
                # [1,1024] sums row is done spread across 64 partitions (DMA
                # repack) -- a [1,N] DVE reciprocal is ~6.5us.
                srow = npool.tile([1, 1024], F32, tag="srow",
                                  name=f"srow{p}{qb}", bufs=1)
                nc.vector.tensor_copy(srow[:], po[HD:HD + 1, :])
                rp = npool.tile([64, 16], F32, tag="rp", name=f"rp{p}{qb}", bufs=1)
                nc.sync.dma_start(
                    rp[:], srow[0:1, :].rearrange("a (b c) -> a b c", b=64)
                )
                pou = npool.tile([HD, 1024], F32, tag="pou",
                                 name=f"pou{p}{qb}", bufs=2)
                # two halves: po bank 1 frees ~0.6us earlier, unblocking the
                # next block's first AV accumulation
                nc.vector.tensor_copy(pou[:, 0:QB], po[0:HD, 0:QB])
                nc.vector.tensor_copy(pou[:, QB:1024], po[0:HD, QB:1024])
                rr = npool.tile([64, 16], F32, tag="rr", name=f"rr{p}{qb}", bufs=1)
                nc.vector.reciprocal(rr[:], rp[:])
                rs = npool.tile([1, 1024], F32, tag="rs", name=f"rs{p}{qb}", bufs=1)
                nc.sync.dma_start(
                    rs[0:1, :].rearrange("a (b c) -> a b c", b=64), rr[:]
                )
                rb = npool.tile([HD, 1024], F32, tag="rb", name=f"rb{p}{qb}", bufs=1)
                nc.gpsimd.partition_broadcast(rb[:], rs[:])
                on = npool.tile([128, QB], BF, tag="on", name=f"on{p}{qb}", bufs=8)
                nc.vector.tensor_mul(on[0:HD, :], pou[0:HD, 0:QB], rb[:, 0:QB])
                nc.vector.tensor_mul(on[HD:128, :], pou[0:HD, QB:1024],
                                     rb[:, QB:1024])
                on_tiles[(p, qb)] = on

            _wo_state = {}

            def wo_unit(qb, qt, oc):
                on0 = on_tiles[(0, qb)]
                on1 = on_tiles[(1, qb)]
                tqs = slice(qt * 128, (qt + 1) * 128)
                ti = qb * (QB // 128) + qt
                if oc == 0:
                    ysb = ypool.tile([128, H], BF, tag="y", name=f"y{qb}{qt}")
                    _wo_state[(qb, qt)] = ysb
                else:
                    ysb = _wo_state.pop((qb, qt))
                ocs = slice(oc * 512, (oc + 1) * 512)
                py = psum.tile([128, 512], F32, tag="py",
                               name=f"py{qb}{qt}{oc}", bufs=2)
                nc.tensor.matmul(
                    py[:], lhsT=on0[:, tqs], rhs=wo_sb[:, 0, ocs],
                    start=True, stop=False,
                )
                nc.tensor.matmul(
                    py[:], lhsT=on1[:, tqs], rhs=wo_sb[:, 1, ocs],
                    start=False, stop=True,
                )
                nc.vector.tensor_copy(ysb[:, ocs], py[:])
                if oc == 1:
                    nc.sync.dma_start(y_ap[ti], ysb[:])

            def wo_units(qb):
                return [
                    (lambda qt=qt, oc=oc: wo_unit(qb, qt, oc))
                    for qt in range(QB // 128) for oc in (0, 1)
                ]

            def wo_block(qb):
                for u in wo_units(qb):
                    u()

            # --- emission order == scheduling priority ---
            # minimal serial intro: just enough for attention(0,0) scores
            qk_proj(0, xq_t, wq_sb, bq_sb, QT[0], wins=[0])
            qk_proj(0, xk_t, wk_sb, bk_sb, KT[0], wins=[0])

            # everything else rides the filler queue, ordered so each unit
            # lands before its first consumer; qb0 runs scores/exp first
            # (split) so ACT starts early, with its AV matmuls as phase B
            filler += proj_units(0, xk_t, wk_sb, bk_sb, KT[0], [1, 2, 3])
            filler += proj_units(0, xq_t, wq_sb, bq_sb, QT[0], [1])
            filler += v_units(range(0, 16))
            attention(0, 0, split=True, pops=1)
            filler += proj_units(0, xq_t, wq_sb, bq_sb, QT[0], [2])
            filler += proj_units(1, xk_t, wk_sb, bk_sb, KT[1], [0, 1])
            attention(0, 1)
            filler += proj_units(0, xq_t, wq_sb, bq_sb, QT[0], [3])
            filler += proj_units(1, xk_t, wk_sb, bk_sb, KT[1], [2, 3])
            attention(0, 2)
            filler += proj_units(1, xq_t, wq_sb, bq_sb, QT[1], [0])
            filler += proj_units(1, xq_t, wq_sb, bq_sb, QT[1], [1])
            attention(0, 3)
            filler += proj_units(1, xq_t, wq_sb, bq_sb, QT[1], [2])
            attention(1, 0)
            filler += wo_units(0)
            filler += proj_units(1, xq_t, wq_sb, bq_sb, QT[1], [3])
            attention(1, 1)
            filler += wo_units(1)
            attention(1, 2)
            attention(1, 3)
            while filler:
                filler.pop(0)()
            # wo(2) lands here so its matmuls fill the PE gap while the last
            # block's normalization chain runs
            wo_block(2)
            wo_block(3)
    nc.compile()
    return nc


def _get_nc():
    global _nc
    with _cache:
        if _nc is None:
            _nc = _build_nc()
        return _nc


def kernel(q, k, v, wq_w, wq_b, wk_w, wk_b, wv_w, wv_b, wo_w, wo_b):
    global LAST_RESULT
    nc = _get_nc()

    def xT(a, b):
        return np.ascontiguousarray(np.asarray(a)[b].astype(BF16).T)

    wq_w = np.asarray(wq_w, dtype=np.float32)
    wk_w = np.asarray(wk_w, dtype=np.float32)
    wv_w = np.asarray(wv_w, dtype=np.float32)
    wo_w = np.asarray(wo_w, dtype=np.float32)

    xs = {}
    for b in range(B):
        xs[b] = (xT(q, b), xT(k, b), xT(v, b))

    in_maps = []
    for c in range(N_CORES):
        b, g = c // NG, c % NG
        cs = slice(g * CPG, (g + 1) * CPG)
        xq_t, xk_t, xv_t = xs[b]
        in_maps.append({
            "xq_t": xq_t,
            "xk_t": xk_t,
            "xv_t": xv_t,
            "wq_t": np.ascontiguousarray(wq_w[cs, :].astype(BF16).T),
            "wk_t": np.ascontiguousarray(wk_w[cs, :].astype(BF16).T),
            "wv_t": np.ascontiguousarray(wv_w[cs, :].astype(BF16).T),
            "bq": np.asarray(wq_b, np.float32)[cs].reshape(CPG, 1).copy(),
            "bk": np.asarray(wk_b, np.float32)[cs].reshape(CPG, 1).copy(),
            "bv": np.asarray(wv_b, np.float32)[cs].reshape(1, CPG).copy(),
            "wo_t": np.ascontiguousarray(wo_w[:, cs].astype(BF16).T),
        })

    res = run_bass_kernel_spmd(
        nc, in_maps, core_ids=list(range(N_CORES)),
        trace=bool(int(os.environ.get("MHA_TRACE", "0"))),
    )
    LAST_RESULT = res

    out = np.empty((B, S, H), dtype=np.float32)
    wo_bias = np.asarray(wo_b, np.float32)[None, :]
    for b in range(B):
        acc = res.results[b * NG]["y_t"].astype(np.float32)
        for g in range(1, NG):
            acc += res.results[b * NG + g]["y_t"].astype(np.float32)
        out[b] = acc + wo_bias
    return out


# revision 15
# speedup vs baseline: 1.0463x; 1.0463x over previous
"""Multi-head attention (B=2, S=2048, H=1024, 16 heads) on 8 trn2 NeuronCores.

Sharding: batch(2) x head-group(4) tensor parallel. Core (b, g) owns batch b
and heads 4g..4g+3 (channels 256g..256g+256 of the QKV projections / input
channels of the output projection). Partial wo outputs are summed on host.

Device-side dataflow per core (matmuls bf16, f32 PSUM accumulation):
  QT/KT[c, s]: transposed projections (channels on partitions); bias fused
  into the PSUM-drain copy (tensor_scalar_add with a [128,1] bias column).
  V projected directly in natural layout [s, ch] (lhsT = x chunk), drained
  with a strided scalar_tensor_tensor into VA4[s, kk, head, 65] (col 64 is
  a ones column -> row sums ride the AV matmul).
  Per head-pair p, query-block qb (512 q), key-tile kk (128 k):
    sc[k, 0:512]=h_even scores, sc[k, 512:1024]=h_odd  (row-packed concurrent)
    e = exp(sc/8)  (single [128,1024] ACT instr, both heads)
    po[0:65, 0:512] += VA4[.,kk,even,:] . e_even ; po[:, 512:1024] += odd
  Epilogue: early-drain po->SBUF, DMA-repack of the sums row to [64,16] for
  a cheap DVE reciprocal, DMA back, gpsimd partition_broadcast, DVE muls.
  wo flipped: y[q, oc] = on_pair0.T @ wo0 + on_pair1.T @ wo1.
  Intro: DMA descriptor issue spread across all 5 engine queues (it
  serializes ~650ns/descriptor per queue); 12 warmup matmuls keep the PE
  HAM clock-gate warm before real work lands; wo(2) rides inside the last
  attention block so its normalization chain hides under matmuls.
"""

import os
import threading

import numpy as np
import ml_dtypes

import concourse.bass as bass
import concourse.mybir as mybir
import concourse.tile as tile
from concourse import bacc
from concourse.bass_utils import run_bass_kernel_spmd

BF16 = ml_dtypes.bfloat16
F32 = mybir.dt.float32
BF = mybir.dt.bfloat16

B = 2
S = 2048
H = 1024
NH = 16
HD = 64
NG = 4              # head groups (TP degree)
HPG = 4             # heads per group
CPG = HPG * HD      # 256 channels per group
NF = H // 128       # 8 input-feature chunks
N_CORES = 8
NKT = S // 128      # 16 key tiles
NQB = S // 512      # 4 query blocks
QB = 512

_cache = threading.Lock()
_nc = None

LAST_RESULT = None  # BassKernelResults of the most recent run (for test.py)


def _build_nc():
    nc = bacc.Bacc(None, target_bir_lowering=False, debug=False)

    xq_d = nc.dram_tensor("xq_t", [H, S], BF, kind="ExternalInput")
    xk_d = nc.dram_tensor("xk_t", [H, S], BF, kind="ExternalInput")
    xv_d = nc.dram_tensor("xv_t", [H, S], BF, kind="ExternalInput")
    wq_d = nc.dram_tensor("wq_t", [H, CPG], BF, kind="ExternalInput")
    wk_d = nc.dram_tensor("wk_t", [H, CPG], BF, kind="ExternalInput")
    wv_d = nc.dram_tensor("wv_t", [H, CPG], BF, kind="ExternalInput")
    bqk_d = nc.dram_tensor("bqk", [CPG, 2], F32, kind="ExternalInput")
    bv_d = nc.dram_tensor("bv", [1, CPG], F32, kind="ExternalInput")
    wo_d = nc.dram_tensor("wo_t", [CPG, H], BF, kind="ExternalInput")
    y_d = nc.dram_tensor("y_t", [S, H], BF, kind="ExternalOutput")

    xq_ap = xq_d.rearrange("(nf p) s -> nf p s", p=128)
    xk_ap = xk_d.rearrange("(nf p) s -> nf p s", p=128)
    xv_ap = xv_d.rearrange("(nf p) s -> nf p s", p=128)
    y_ap = y_d.rearrange("(nt p) o -> nt p o", p=128)

    Exp = mybir.ActivationFunctionType.Exp
    MULT = mybir.AluOpType.mult
    ADD = mybir.AluOpType.add

    with tile.TileContext(nc) as tc:
        with (
            tc.tile_pool(name="const", bufs=1) as const,
            tc.tile_pool(name="xpool", bufs=1) as xpool,
            tc.tile_pool(name="res", bufs=1) as res,
            tc.tile_pool(name="epool", bufs=2) as epool,
            tc.tile_pool(name="npool", bufs=2) as npool,
            tc.tile_pool(name="ypool", bufs=3) as ypool,
            tc.tile_pool(name="psum", bufs=1, space="PSUM") as psum,
        ):
            # --- constants / weights ---
            wq_sb = const.tile([128, NF, CPG], BF)
            wk_sb = const.tile([128, NF, CPG], BF)
            wv_sb = const.tile([128, NF, CPG], BF)
            wo_sb = const.tile([128, 2, H], BF)
            bqk_sb = const.tile([128, 2, 2], F32)
            bv_sb = const.tile([1, CPG], F32)
            bvb = const.tile([128, CPG], F32)
            warm = const.tile([128, QB], BF)

            # --- residents ---
            QT = [res.tile([128, S], BF, name=f"QT{p}") for p in range(2)]
            KT = [res.tile([128, S], BF, name=f"KT{p}") for p in range(2)]
            VA4 = res.tile([128, NKT, 4, HD + 1], BF, name="VA4")

            # --- input staging ---
            # Each dma_start costs ~2us of fixed completion latency on top of
            # bytes/BW, and transfers serialize per issuing ring (HWDGE: sync,
            # scalar; SWDGE: gpsimd, slower still). So: few, large
            # consolidated descriptors, ordered by need-time, critical path
            # (wq + xq win0 | wk + xk win0) split across the two HWDGE rings.
            xq0_t = xpool.tile([128, NF, QB], BF, tag="xq0")
            xqr_t = xpool.tile([128, NF, 3 * QB], BF, tag="xqr")
            xk0_t = xpool.tile([128, NF, QB], BF, tag="xk0")
            xk1_t = xpool.tile([128, NF, QB], BF, tag="xk1")
            xk23_t = xpool.tile([128, NF, 2 * QB], BF, tag="xk23")
            xvA_t = xpool.tile([128, 4, S], BF, tag="xvA")
            xvB_t = xpool.tile([128, 4, S], BF, tag="xvB")

            xq_pa = xq_d.rearrange("(nf p) s -> p nf s", p=128)
            xk_pa = xk_d.rearrange("(nf p) s -> p nf s", p=128)
            xv_pa = xv_d.rearrange("(nf p) s -> p nf s", p=128)

            # gpsimd (SWDGE, slow): memsets, V-side weights, wo
            nc.gpsimd.memset(warm[:], 0.125)
            nc.gpsimd.memset(VA4[:, :, :, HD : HD + 1], 1.0)
            nc.gpsimd.dma_start(wv_sb[:], wv_d.rearrange("(nf p) c -> p nf c", p=128))
            nc.gpsimd.dma_start(bv_sb[:], bv_d[:])
            nc.gpsimd.partition_broadcast(bvb[:], bv_sb[:])
            nc.gpsimd.dma_start(wo_sb[:], wo_d.rearrange("(t p) o -> p t o", p=128))

            # sync ring: biases, wq, then K windows / V / Q-rest by need-time
            nc.sync.dma_start(bqk_sb[:],
                              bqk_d.rearrange("(t p) c -> p t c", p=128))
            nc.sync.dma_start(wq_sb[:], wq_d.rearrange("(nf p) c -> p nf c", p=128))
            nc.sync.dma_start(xk1_t[:], xk_pa[:, :, QB:2 * QB])
            nc.sync.dma_start(xqr_t[:], xq_pa[:, :, QB:S])
            nc.sync.dma_start(xvA_t[:], xv_pa[:, 0:4, :])

            # scalar ring: xq win0 first (first matmul), wk, xk win0, rest
            nc.scalar.dma_start(xq0_t[:], xq_pa[:, :, 0:QB])
            nc.scalar.dma_start(wk_sb[:], wk_d.rearrange("(nf p) c -> p nf c", p=128))
            nc.scalar.dma_start(xk0_t[:], xk_pa[:, :, 0:QB])
            dume = const.tile([1, 16], F32)
            nc.scalar.activation(dume[:], warm[0:1, 0:16], Exp, scale=1.0)
            nc.scalar.dma_start(xk23_t[:], xk_pa[:, :, 2 * QB:S])
            nc.scalar.dma_start(xvB_t[:], xv_pa[:, 4:8, :])

            # --- PE warmup: keep HAM clock-gate busy until real work lands ---
            for i in range(24):
                pw = psum.tile([128, 256], F32, tag="py", name=f"warm{i}", bufs=2)
                nc.tensor.matmul(pw[:], lhsT=warm[:, 0:128], rhs=warm[:, 0:256],
                                 start=True, stop=True)

            def xq_t(f, win):
                if win == 0:
                    return xq0_t[:, f, :]
                return xqr_t[:, f, (win - 1) * QB:win * QB]

            def xk_t(f, win):
                if win == 0:
                    return xk0_t[:, f, :]
                if win == 1:
                    return xk1_t[:, f, :]
                return xk23_t[:, f, (win - 2) * QB:(win - 1) * QB]

            # ---- filler units: ~0.5-1.5us chunks of PE side-work that get
            # interleaved into the attention kk-loops (the static scheduler
            # follows emission order per engine, so coarse blocks stall ACT)
            _proj_state = {}

            def proj_half(p, xt, w_sb, bi, out_t, win, half):
                cs = slice(p * 128, (p + 1) * 128)
                ws = slice(win * QB, (win + 1) * QB)
                key = (id(out_t), win)
                if half == 0:
                    ps = psum.tile([128, QB], F32, tag="py",
                                   name=f"ps{p}{win}", bufs=2)
                    _proj_state[key] = ps
                    fs = range(0, NF // 2)
                else:
                    ps = _proj_state.pop(key)
                    fs = range(NF // 2, NF)
                for f in fs:
                    nc.tensor.matmul(
                        ps[:], lhsT=w_sb[:, f, cs], rhs=xt(f, win),
                        start=(f == 0), stop=(f == NF - 1),
                    )
                if half == 1:
                    nc.vector.tensor_scalar_add(
                        out_t[:, ws], ps[:], bqk_sb[:, p, bi:bi + 1])

            def qk_proj(p, xt, w_sb, bi, out_t, wins=None):
                for win in (range(S // QB) if wins is None else wins):
                    proj_half(p, xt, w_sb, bi, out_t, win, 0)
                    proj_half(p, xt, w_sb, bi, out_t, win, 1)

            def proj_units(p, xt, w_sb, bi, out_t, wins):
                return [
                    (lambda w=w, h=h: proj_half(p, xt, w_sb, bi, out_t, w, h))
                    for w in wins for h in (0, 1)
                ]

            # V in natural layout: pv[s, ch] = sum_f xv[f,s].T @ wv[f,ch],
            # drained (plus bias) straight into the augmented VA4 tile.
            def v_unit(st):
                tsl = slice(st * 128, (st + 1) * 128)
                pv = psum.tile([128, CPG], F32, tag="py",
                               name=f"pv{st}", bufs=2)
                for f in range(NF):
                    nc.tensor.matmul(
                        pv[:],
                        lhsT=(xvA_t[:, f, tsl] if f < 4 else
                              xvB_t[:, f - 4, tsl]),
                        rhs=wv_sb[:, f, :],
                        start=(f == 0), stop=(f == NF - 1),
                    )
                nc.vector.scalar_tensor_tensor(
                    VA4[:, st, :, 0:HD],
                    pv[:].rearrange("p (h d) -> p h d", h=4),
                    1.0,
                    bvb[:].rearrange("p (h d) -> p h d", h=4),
                    MULT, ADD,
                )

            def v_units(sts):
                return [(lambda s=s: v_unit(s)) for s in sts]

            on_tiles = {}
            filler = []

            def self_av(p, po, kk, e):
                nc.tensor.matmul(
                    po[0:HD + 1, 0:QB], lhsT=VA4[:, kk, 2 * p, :],
                    rhs=e[:, 0:QB],
                    start=(kk == 0), stop=(kk == NKT - 1),
                )
                nc.tensor.matmul(
                    po[0:HD + 1, QB:1024], lhsT=VA4[:, kk, 2 * p + 1, :],
                    rhs=e[:, QB:1024],
                    start=(kk == 0), stop=(kk == NKT - 1),
                )

            def attention(p, qb, split=False, pops=2):
                qs = slice(qb * QB, (qb + 1) * QB)
                po = psum.tile([128, 1024], F32, tag="po",
                               name=f"po{p}{qb}", bufs=1)
                es = []
                for kk in range(NKT):
                    for _ in range(pops):
                        if filler:
                            filler.pop(0)()
                    ks = slice(kk * 128, (kk + 1) * 128)
                    sc = psum.tile([128, 1024], F32, tag="sc",
                                   name=f"sc{p}{qb}{kk}", bufs=2)
                    nc.tensor.matmul(
                        sc[:, 0:QB],
                        lhsT=KT[p][0:HD, ks], rhs=QT[p][0:HD, qs],
                        tile_position=(0, 0),
                    )
                    nc.tensor.matmul(
                        sc[:, QB:1024],
                        lhsT=KT[p][HD:128, ks], rhs=QT[p][HD:128, qs],
                        tile_position=(64, 0),
                    )
                    e = epool.tile([128, 1024], BF, tag="e", name=f"e{p}{qb}{kk}",
                                   bufs=16)
                    nc.scalar.activation(e[:], sc[:], Exp, scale=0.125)
                    es.append(e)
                    if not split:
                        self_av(p, po, kk, es[kk])
                if split:
                    for kk in range(NKT):
                        if filler:
                            filler.pop(0)()
                        self_av(p, po, kk, es[kk])
                # epilogue: drain the sums row first (it gates the whole
                # reciprocal round-trip), then the po body; reciprocal of the
                # [1,1024] sums row is done spread across 64 partitions (DMA
                # repack) -- a [1,N] DVE reciprocal is ~6.5us.
                srow = npool.tile([1, 1024], F32, tag="srow",
                                  name=f"srow{p}{qb}", bufs=1)
                nc.vector.tensor_copy(srow[:], po[HD:HD + 1, :])
                rp = npool.tile([64, 16], F32, tag="rp", name=f"rp{p}{qb}", bufs=1)
                nc.sync.dma_start(
                    rp[:], srow[0:1, :].rearrange("a (b c) -> a b c", b=64)
                )
                pou = npool.tile([HD, 1024], F32, tag="pou",
                                 name=f"pou{p}{qb}", bufs=2)
                # two halves: po bank 1 frees ~0.6us earlier, unblocking the
                # next block's first AV accumulation
                nc.vector.tensor_copy(pou[:, 0:QB], po[0:HD, 0:QB])
                nc.vector.tensor_copy(pou[:, QB:1024], po[0:HD, QB:1024])
                rr = npool.tile([64, 16], F32, tag="rr", name=f"rr{p}{qb}", bufs=1)
                nc.vector.reciprocal(rr[:], rp[:])
                rs = npool.tile([1, 1024], F32, tag="rs", name=f"rs{p}{qb}", bufs=1)
                nc.sync.dma_start(
                    rs[0:1, :].rearrange("a (b c) -> a b c", b=64), rr[:]
                )
                rb = npool.tile([HD, 1024], F32, tag="rb", name=f"rb{p}{qb}", bufs=1)
                nc.gpsimd.partition_broadcast(rb[:], rs[:])
                on = npool.tile([128, QB], BF, tag="on", name=f"on{p}{qb}", bufs=8)
                nc.vector.tensor_mul(on[0:HD, :], pou[0:HD, 0:QB], rb[:, 0:QB])
                nc.vector.tensor_mul(on[HD:128, :], pou[0:HD, QB:1024],
                                     rb[:, QB:1024])
                on_tiles[(p, qb)] = on

            _wo_state = {}

            def wo_unit(qb, qt, oc):
                on0 = on_tiles[(0, qb)]
                on1 = on_tiles[(1, qb)]
                tqs = slice(qt * 128, (qt + 1) * 128)
                ti = qb * (QB // 128) + qt
                if oc == 0:
                    ysb = ypool.tile([128, H], BF, tag="y", name=f"y{qb}{qt}")
                    _wo_state[(qb, qt)] = ysb
                else:
                    ysb = _wo_state.pop((qb, qt))
                ocs = slice(oc * 512, (oc + 1) * 512)
                py = psum.tile([128, 512], F32, tag="py",
                               name=f"py{qb}{qt}{oc}", bufs=2)
                nc.tensor.matmul(
                    py[:], lhsT=on0[:, tqs], rhs=wo_sb[:, 0, ocs],
                    start=True, stop=False,
                )
                nc.tensor.matmul(
                    py[:], lhsT=on1[:, tqs], rhs=wo_sb[:, 1, ocs],
                    start=False, stop=True,
                )
                nc.vector.tensor_copy(ysb[:, ocs], py[:])
                if oc == 1:
                    nc.sync.dma_start(y_ap[ti], ysb[:])

            def wo_units(qb):
                return [
                    (lambda qt=qt, oc=oc: wo_unit(qb, qt, oc))
                    for qt in range(QB // 128) for oc in (0, 1)
                ]

            def wo_block(qb):
                for u in wo_units(qb):
                    u()

            # --- emission order == scheduling priority ---
            # minimal serial intro: just enough for attention(0,0) scores
            qk_proj(0, xq_t, wq_sb, 0, QT[0], wins=[0])
            qk_proj(0, xk_t, wk_sb, 1, KT[0], wins=[0])

            # everything else rides the filler queue, ordered so each unit
            # lands before its first consumer; qb0 runs scores/exp first
            # (split) so ACT starts early, with its AV matmuls as phase B
            filler += proj_units(0, xk_t, wk_sb, 1, KT[0], [1, 2, 3])
            filler += proj_units(0, xq_t, wq_sb, 0, QT[0], [1])
            filler += v_units(range(0, 16))
            attention(0, 0, split=True, pops=1)
            filler += proj_units(0, xq_t, wq_sb, 0, QT[0], [2])
            filler += proj_units(1, xk_t, wk_sb, 1, KT[1], [0, 1])
            attention(0, 1)
            filler += proj_units(0, xq_t, wq_sb, 0, QT[0], [3])
            filler += proj_units(1, xk_t, wk_sb, 1, KT[1], [2, 3])
            attention(0, 2)
            filler += proj_units(1, xq_t, wq_sb, 0, QT[1], [0])
            filler += proj_units(1, xq_t, wq_sb, 0, QT[1], [1])
            attention(0, 3)
            filler += proj_units(1, xq_t, wq_sb, 0, QT[1], [2])
            attention(1, 0)
            filler += wo_units(0)
            filler += proj_units(1, xq_t, wq_sb, 0, QT[1], [3])
            attention(1, 1)
            filler += wo_units(1)
            attention(1, 2)
            attention(1, 3)
            while filler:
                filler.pop(0)()
            # wo(2) lands here so its matmuls fill the PE gap while the last
            # block's normalization chain runs
            wo_block(2)
            wo_block(3)
    nc.compile()
    return nc


def _get_nc():
    global _nc
    with _cache:
        if _nc is None:
            _nc = _build_nc()
        return _nc


def kernel(q, k, v, wq_w, wq_b, wk_w, wk_b, wv_w, wv_b, wo_w, wo_b):
    global LAST_RESULT
    nc = _get_nc()

    def xT(a, b):
        return np.ascontiguousarray(np.asarray(a)[b].astype(BF16).T)

    wq_w = np.asarray(wq_w, dtype=np.float32)
    wk_w = np.asarray(wk_w, dtype=np.float32)
    wv_w = np.asarray(wv_w, dtype=np.float32)
    wo_w = np.asarray(wo_w, dtype=np.float32)

    xs = {}
    for b in range(B):
        xs[b] = (xT(q, b), xT(k, b), xT(v, b))

    in_maps = []
    for c in range(N_CORES):
        b, g = c // NG, c % NG
        cs = slice(g * CPG, (g + 1) * CPG)
        xq_t, xk_t, xv_t = xs[b]
        in_maps.append({
            "xq_t": xq_t,
            "xk_t": xk_t,
            "xv_t": xv_t,
            "wq_t": np.ascontiguousarray(wq_w[cs, :].astype(BF16).T),
            "wk_t": np.ascontiguousarray(wk_w[cs, :].astype(BF16).T),
            "wv_t": np.ascontiguousarray(wv_w[cs, :].astype(BF16).T),
            "bqk": np.stack(
                [np.asarray(wq_b, np.float32)[cs],
                 np.asarray(wk_b, np.float32)[cs]], axis=1).copy(),
            "bv": np.asarray(wv_b, np.float32)[cs].reshape(1, CPG).copy(),
            "wo_t": np.ascontiguousarray(wo_w[:, cs].astype(BF16).T),
        })

    res = run_bass_kernel_spmd(
        nc, in_maps, core_ids=list(range(N_CORES)),
        trace=bool(int(os.environ.get("MHA_TRACE", "0"))),
    )
    LAST_RESULT = res

    out = np.empty((B, S, H), dtype=np.float32)
    wo_bias = np.asarray(wo_b, np.float32)[None, :]
    for b in range(B):
        acc = res.results[b * NG]["y_t"].astype(np.float32)
        for g in range(1, NG):
            acc += res.results[b * NG + g]["y_t"].astype(np.float32)
        out[b] = acc + wo_bias
    return out
